# revision 19
# baseline (speedup 1.0000x reference)
"""Trainium2 Bass kernel for nn_CustomRelation (sparse_attention).

Computation (per batch b):
    qkw = hidden @ W + bias                      # [S, 128] = [q(64) | k(64)]
    rope(qkw) with interleaved sin/cos           # RoPE on both halves
    logits[r] = q[i0[r]] . k[i2[r]] + q[i1[r]] . k[i3[r]]
    out = (logits + (1 - mask) * -1e12) / 8

Distribution: data-parallel over batch, 2 batches per NeuronCore x 8 cores.

Per-core dataflow (feature-major):
  - X [4096, 1024] loaded in [128, 1024] tiles; PE-transposed (vs identity)
    into XT [128h, 512t] tiles (PSUM -> SBUF copies split over ACT/DVE).
  - qkw^T [128f, 512t] = sum_k W_k^T @ XT_k  (PE, accumulate in PSUM).
  - rot(qkw) via a signed pair-swap permutation matmul (PE).
  - sin/cos tables computed on device from position_ids:
    pg = pos * invf/(2pi) (K=1 outer-product matmul), f = pg - round(pg)
    (round via the +1.5*2^23 magic trick), sin = Sin(2pi*f),
    cos = Sin(-2pi*|f| + pi/2)  (ACT Sin valid range is [-pi, pi]).
  - rope = qkw*cos + rot*sin (DVE/GPSIMD), PE-transposed back to token-major
    and stored to an internal DRAM table [2048, 128] per batch.
  - 4 indirect-DMA gathers per batch (row gather by relation endpoints),
    fused mul+reduce dot products, mask, scale, store.
"""

import numpy as np

B, S, H, D = 16, 2048, 1024, 64
NCORES = 8
BC = B // NCORES            # batches per core
T = BC * S                  # tokens per core
DH = 2 * D                  # projected features (q|k)
MT = 512                    # macro-tile tokens
NM = T // MT                # macro-tiles per core
NG = MT // 128              # 128-token groups per macro-tile
KH = H // 128               # contraction chunks
RPP = S // 128              # relations per partition (per batch)
MAGIC = 1.5 * 2.0 ** 23
TWO_PI = float(2.0 * np.pi)
HALF_PI = float(0.5 * np.pi)

_CACHE = {}
ADD_TAB_DEPS = True
DEBUG_TABS = False


def _build_nc():
    import concourse.bass as bass
    import concourse.tile as tile
    from concourse import bacc, mybir

    f32 = mybir.dt.float32
    i32 = mybir.dt.int32
    Alu = mybir.AluOpType
    Act = mybir.ActivationFunctionType

    nc = bacc.Bacc("TRN2", target_bir_lowering=False, debug=False,
                   num_devices=NCORES)

    x = nc.dram_tensor("x", [T, H], f32, kind="ExternalInput")
    w = nc.dram_tensor("w", [H, DH], f32, kind="ExternalInput")
    bvec = nc.dram_tensor("bvec", [DH], f32, kind="ExternalInput")
    pos = nc.dram_tensor("pos", [BC, S], i32, kind="ExternalInput")
    rel = nc.dram_tensor("rel", [BC, S, 4], i32, kind="ExternalInput")
    msk = nc.dram_tensor("msk", [BC, S], f32, kind="ExternalInput")
    out = nc.dram_tensor("out", [BC, S], f32, kind="ExternalOutput")

    # RoPE'd [q|k] rows, token-major, one table per batch (gather source).
    tab_kind = "ExternalOutput" if DEBUG_TABS else "Internal"
    tabs = [nc.dram_tensor(f"tab{b}", [S, DH], f32, kind=tab_kind)
            for b in range(BC)]

    # --- constants baked into the NEFF ---
    ident_np = np.eye(128, dtype=np.float32)
    invf = np.power(10000.0, -np.arange(D // 2, dtype=np.float64) / (D / 2.0))
    invf_rep = np.repeat(invf, 2)                      # [64], per q feature
    g_rep = np.concatenate([invf_rep, invf_rep]) / (2 * np.pi)
    g_rep = g_rep.reshape(1, DH).astype(np.float32)    # [1, 128] lhsT (K=1)
    pswapT = np.zeros((DH, DH), dtype=np.float32)      # lhsT of pair swap
    for j in range(D):
        pswapT[2 * j + 1, 2 * j] = -1.0                # rot[2j]   = -x[2j+1]
        pswapT[2 * j, 2 * j + 1] = 1.0                 # rot[2j+1] =  x[2j]
    ident_t = nc.inline_tensor(ident_np, "ident")
    grep_t = nc.inline_tensor(g_rep, "grep")
    pswap_t = nc.inline_tensor(pswapT, "pswapT")

    with tile.TileContext(nc) as tc, \
         tc.tile_pool(name="consts", bufs=1) as consts, \
         tc.tile_pool(name="xp", bufs=2) as xp, \
         tc.tile_pool(name="xt", bufs=2) as xtp, \
         tc.tile_pool(name="sb", bufs=2) as sbp, \
         tc.tile_pool(name="sc", bufs=2) as scp, \
         tc.tile_pool(name="gth", bufs=1) as gth, \
         tc.tile_pool(name="ps_xt", bufs=2, space="PSUM") as ps_xt, \
         tc.tile_pool(name="ps_qkw", bufs=2, space="PSUM") as ps_qkw, \
         tc.tile_pool(name="ps_rot", bufs=1, space="PSUM") as ps_rot, \
         tc.tile_pool(name="ps_pg", bufs=1, space="PSUM") as ps_pg, \
         tc.tile_pool(name="ps_ott", bufs=1, space="PSUM") as ps_ott:

        # ---- constants / small inputs to SBUF ----
        ident = consts.tile([128, 128], f32, tag="ident")
        nc.sync.dma_start(out=ident[:], in_=ident_t.ap())
        grep = consts.tile([1, DH], f32, tag="grep")
        nc.sync.dma_start(out=grep[:], in_=grep_t.ap())
        pswap = consts.tile([DH, DH], f32, tag="pswap")
        nc.sync.dma_start(out=pswap[:], in_=pswap_t.ap())
        bcol = consts.tile([DH, 1], f32, tag="bcol")
        nc.sync.dma_start(out=bcol[:], in_=bvec.ap().rearrange("(p o) -> p o", o=1))
        wsb = []
        for k in range(KH):
            wk = consts.tile([128, DH], f32, tag=f"w{k}")
            nc.sync.dma_start(out=wk[:], in_=w[128 * k:128 * (k + 1), :])
            wsb.append(wk)
        pos_fb = []
        for b in range(BC):
            pib = consts.tile([1, S], i32, tag=f"pos_i{b}")
            nc.sync.dma_start(out=pib[:], in_=pos[b:b + 1, :])
            pfb = consts.tile([1, S], f32, tag=f"pos_f{b}")
            nc.vector.tensor_copy(out=pfb[:], in_=pib[:])
            pos_fb.append(pfb)
        zerob = consts.tile([DH, 1], f32, tag="zerob")
        nc.vector.memset(zerob[:], 0.0)
        pihalf = consts.tile([DH, 1], f32, tag="pihalf")
        nc.vector.memset(pihalf[:], HALF_PI)

        tab_store_insts = [[] for _ in range(BC)]

        # ---- main pipeline over macro-tiles ----
        for m in range(NM):
            bi = m // (NM // BC)
            t0 = m * MT                    # global token offset (core-local)
            tl0 = t0 - bi * S              # batch-local token offset

            # load X tiles [128, 1024]
            xtl = []
            for g in range(NG):
                xg = xp.tile([128, H], f32, tag=f"x{g}")
                nc.sync.dma_start(out=xg[:], in_=x[t0 + 128 * g:t0 + 128 * (g + 1), :])
                xtl.append(xg)

            # transpose to XT [128h, 512t] per k-chunk; assemble in PSUM
            xts = []
            for k in range(KH):
                pst = ps_xt.tile([128, MT], f32, tag="xtps")
                for g in range(NG):
                    nc.tensor.transpose(
                        out=pst[:, 128 * g:128 * (g + 1)],
                        in_=xtl[g][:, 128 * k:128 * (k + 1)],
                        identity=ident[:],
                    )
                xk = xtp.tile([128, MT], f32, tag=f"xt{k}")
                if k < 3:
                    nc.scalar.copy(out=xk[:], in_=pst[:])
                else:
                    nc.vector.tensor_copy(out=xk[:], in_=pst[:])
                xts.append(xk)

            # qkw^T [128f, 512t] = sum_k W_k^T @ XT_k   (+ bias)
            pq = ps_qkw.tile([DH, MT], f32, tag="qkw")
            for k in range(KH):
                nc.tensor.matmul(out=pq[:], lhsT=wsb[k][:], rhs=xts[k][:],
                                 start=(k == 0), stop=(k == KH - 1))
            qkw = sbp.tile([DH, MT], f32, tag="qkw_sb")
            nc.vector.tensor_scalar(out=qkw[:], in0=pq[:], scalar1=bcol[:],
                                    scalar2=None, op0=Alu.add)

            # rot(qkw) via signed pair swap
            pr = ps_rot.tile([DH, MT], f32, tag="rot")
            nc.tensor.matmul(out=pr[:], lhsT=pswap[:], rhs=qkw[:],
                             start=True, stop=True)

            # sin/cos tables for these positions
            pg = ps_pg.tile([DH, MT], f32, tag="pg")
            nc.tensor.matmul(out=pg[:], lhsT=grep[:],
                             rhs=pos_fb[bi][:, tl0:tl0 + MT],
                             start=True, stop=True)
            tmag = sbp.tile([DH, MT], f32, tag="tmag")
            nc.scalar.activation(out=tmag[:], in_=pg[:], func=Act.Copy,
                                 bias=MAGIC)
            mrnd = sbp.tile([DH, MT], f32, tag="mrnd")
            nc.vector.tensor_scalar(out=mrnd[:], in0=tmag[:], scalar1=-MAGIC,
                                    scalar2=None, op0=Alu.add)
            frac = sbp.tile([DH, MT], f32, tag="frac")
            nc.vector.tensor_tensor(out=frac[:], in0=pg[:], in1=mrnd[:],
                                    op=Alu.subtract)
            sint = scp.tile([DH, MT], f32, tag="sin")
            nc.scalar.activation(out=sint[:], in_=frac[:], func=Act.Sin,
                                 scale=TWO_PI, bias=zerob[:])
            afrac = sbp.tile([DH, MT], f32, tag="afrac")
            nc.scalar.activation(out=afrac[:], in_=frac[:], func=Act.Abs,
                                 bias=zerob[:])
            cost = scp.tile([DH, MT], f32, tag="cos")
            nc.scalar.activation(out=cost[:], in_=afrac[:], func=Act.Sin,
                                 scale=-TWO_PI, bias=pihalf[:])

            # rope = qkw*cos + rot*sin
            t1 = sbp.tile([DH, MT], f32, tag="t1")
            nc.vector.tensor_tensor(out=t1[:], in0=qkw[:], in1=cost[:],
                                    op=Alu.mult)
            t2 = sbp.tile([DH, MT], f32, tag="t2")
            nc.vector.tensor_tensor(out=t2[:], in0=pr[:], in1=sint[:],
                                    op=Alu.mult)
            rope = sbp.tile([DH, MT], f32, tag="rope")
            nc.vector.tensor_tensor(out=rope[:], in0=t1[:], in1=t2[:],
                                    op=Alu.add)

            # back to token-major and store to the gather table
            po = ps_ott.tile([128, MT], f32, tag="ott")
            for g in range(NG):
                nc.tensor.transpose(
                    out=po[:, 128 * g:128 * (g + 1)],
                    in_=rope[:, 128 * g:128 * (g + 1)],
                    identity=ident[:],
                )
            tok = sbp.tile([128, MT], f32, tag="tok")
            nc.scalar.copy(out=tok[:], in_=po[:])
            dst = tabs[bi][tl0:tl0 + MT, :].rearrange("(g p) f -> p g f", p=128)
            st = nc.sync.dma_start(
                out=dst, in_=tok[:].rearrange("p (g f) -> p g f", f=DH))
            tab_store_insts[bi].append(st)

            # ---- per-batch tail: gathers + dots ----
            if m % (NM // BC) == (NM // BC) - 1:
                relb = gth.tile([128, RPP * 4], i32, tag="relb")
                nc.sync.dma_start(
                    out=relb[:],
                    in_=rel[bi].rearrange("(p j) g -> p (j g)", p=128))
                mskb = gth.tile([128, RPP], f32, tag="mskb")
                nc.sync.dma_start(
                    out=mskb[:], in_=msk[bi].rearrange("(p j) -> p j", p=128))

                dots = []
                for pair, (gq, gk) in enumerate(((0, 2), (1, 3))):
                    gq_t = gth.tile([128, RPP * D], f32, tag=f"gq{pair}")
                    gk_t = gth.tile([128, RPP * D], f32, tag=f"gk{pair}")
                    for j in range(RPP):
                        gi = nc.gpsimd.indirect_dma_start(
                            out=gq_t[:, j * D:(j + 1) * D], out_offset=None,
                            in_=tabs[bi].ap(),
                            in_offset=bass.IndirectOffsetOnAxis(
                                ap=relb[:, 4 * j + gq:4 * j + gq + 1], axis=0),
                            element_offset=0)
                        gi2 = nc.gpsimd.indirect_dma_start(
                            out=gk_t[:, j * D:(j + 1) * D], out_offset=None,
                            in_=tabs[bi].ap(),
                            in_offset=bass.IndirectOffsetOnAxis(
                                ap=relb[:, 4 * j + gk:4 * j + gk + 1], axis=0),
                            element_offset=D)
                        if ADD_TAB_DEPS:
                            for sti in tab_store_insts[bi]:
                                tile.add_dep_helper(gi.ins, sti.ins,
                                                    reason="tab RAW")
                                tile.add_dep_helper(gi2.ins, sti.ins,
                                                    reason="tab RAW")
                    prod = gth.tile([128, RPP * D], f32, tag=f"prod{pair}")
                    nc.vector.tensor_tensor(out=prod[:], in0=gq_t[:],
                                            in1=gk_t[:], op=Alu.mult)
                    dt = gth.tile([128, RPP], f32, tag=f"dots{pair}")
                    nc.vector.tensor_reduce(
                        out=dt[:],
                        in_=prod[:].rearrange("p (j e) -> p j e", e=D),
                        axis=mybir.AxisListType.X, op=Alu.add)
                    dots.append(dt)

                lg = gth.tile([128, RPP], f32, tag="lg")
                nc.vector.tensor_tensor(out=lg[:], in0=dots[0][:],
                                        in1=dots[1][:], op=Alu.add)
                # (logits + (1-mask)*-1e12)/8 == logits/8 + mask*1.25e11 - 1.25e11
                mt_ = gth.tile([128, RPP], f32, tag="mterm")
                nc.vector.tensor_scalar(out=mt_[:], in0=mskb[:],
                                        scalar1=1.25e11, scalar2=-1.25e11,
                                        op0=Alu.mult, op1=Alu.add)
                fin = gth.tile([128, RPP], f32, tag="fin")
                nc.vector.tensor_scalar(out=fin[:], in0=lg[:], scalar1=0.125,
                                        scalar2=None, op0=Alu.mult)
                nc.vector.tensor_tensor(out=fin[:], in0=fin[:], in1=mt_[:],
                                        op=Alu.add)
                nc.sync.dma_start(
                    out=out[bi].rearrange("(p j) -> p j", p=128), in_=fin[:])

    nc.compile()
    return nc


def _get_nc():
    if "nc" not in _CACHE:
        _CACHE["nc"] = _build_nc()
    return _CACHE["nc"]


def _shard(inputs):
    lhs = np.ascontiguousarray(np.asarray(inputs["last_hidden_state"], np.float32))
    w = np.ascontiguousarray(np.asarray(inputs["W"], np.float32))
    b = np.ascontiguousarray(np.asarray(inputs["b"], np.float32))
    pos = np.ascontiguousarray(np.asarray(inputs["position_ids"], np.int32))
    rel = np.ascontiguousarray(np.asarray(inputs["relations_idx"], np.int32))
    msk = np.ascontiguousarray(np.asarray(inputs["labels_mask"], np.float32))
    in_maps = []
    for c in range(NCORES):
        sl = slice(BC * c, BC * (c + 1))
        in_maps.append({
            "x": lhs[sl].reshape(T, H).copy(),
            "w": w,
            "bvec": b,
            "pos": pos[sl].copy(),
            "rel": rel[sl].copy(),
            "msk": msk[sl].copy(),
        })
    return in_maps


def kernel(**inputs):
    from concourse import bass_utils
    nc = _get_nc()
    in_maps = _shard(inputs)
    res = bass_utils.run_bass_kernel_spmd(
        nc, in_maps, core_ids=list(range(NCORES)))
    _CACHE["last_results"] = res
    outs = [res.results[c]["out"].reshape(BC, S) for c in range(NCORES)]
    return np.concatenate(outs, axis=0).astype(np.float32)
